# revision 1
# baseline (speedup 1.0000x reference)
"""Loopy belief propagation on 8 Trainium2 NeuronCores (Bass/Tile).

Strategy (edge-parallel, per the sharding hint):
  - undirected edge pairs are assigned to the 8 cores with per-(node,core)
    balancing; both directions of a pair live on the same core
  - per core, every directed edge (s->j) gets one "in-slot" in node j's row
    of a padded node-major slot array; node j has W(j) = max_c indeg_c(j)
    slots on every core, so all cores share one global layout
  - per iteration: in_log = strided reduce over each node's slots (regular),
    AllReduce [128, N16] node aggregate across cores, then the per-slot
    update  logm' = lognorm(max(exp(max(T[j]-I, lnEPS)), eps) @ psi)
    computed fully regularly in a transposed-stacked layout
    [128 = 16 stripes x 8 classes, cols]; the new message for the reverse
    edge is delivered to its slot by an indirect-DMA row scatter
  - output: belief table, un-permuted on host

All graph-dependent index tables are built on the host (numpy); the device
program is static.
"""
import numpy as np

EPS = 1e-12
LN_EPS = float(np.log(np.float32(EPS)))
N_CORES = 8

# Problem constants (nn_LoopyBP: n=100000, E=3200000, k=8, iterations=5)
N_NODES = 100_000
E_DIR = 3_200_000
K = 8


# --------------------------- host-side layout ---------------------------

def _assign_cores(u_node, v_node, n, rounds=30, seed=0):
    rng = np.random.default_rng(seed)
    H = len(u_node)
    core_of = rng.integers(0, N_CORES, size=H).astype(np.int8)
    u64 = u_node.astype(np.int64)
    v64 = v_node.astype(np.int64)

    def counts(co):
        c1 = np.bincount(u64 * N_CORES + co, minlength=n * N_CORES)
        c2 = np.bincount(v64 * N_CORES + co, minlength=n * N_CORES)
        return (c1 + c2).reshape(n, N_CORES).astype(np.int32)

    for r in range(rounds):
        cnt = counts(core_of)
        tgt = np.ceil(cnt.sum(1) / N_CORES).astype(np.int32)
        over_u = cnt[u64, core_of] > tgt[u64]
        over_v = cnt[v64, core_of] > tgt[v64]
        bad = np.flatnonzero(over_u | over_v)
        if len(bad) == 0:
            break
        load = cnt[u64[bad]] + cnt[v64[bad]]
        newc = np.argmin(load, axis=1).astype(np.int8)
        take = rng.random(len(bad)) < max(0.15, 0.6 * (0.85 ** r))
        core_of[bad[take]] = newc[take]
    return core_of, counts(core_of)


def _build_layout(src, dst, rev_idx, n):
    E = len(src)
    H = E // 2
    rev = rev_idx.astype(np.int64)
    assert np.all(rev[rev] == np.arange(E)), "rev_idx is not an involution"
    assert not np.any(rev == np.arange(E)), "self-reverse edges unsupported"
    firsts = np.flatnonzero(np.arange(E) < rev)
    assert len(firsts) == H
    partner = rev[firsts]
    pu = dst[partner].astype(np.int64)
    pv = dst[firsts].astype(np.int64)

    core_of, cnt = _assign_cores(pu, pv, n)
    W = cnt.max(axis=1)
    assert W.max() <= 128

    order = np.lexsort((np.arange(n), W))
    order = order[W[order] > 0]
    Ws = W[order]
    stripe_of = np.zeros(n, dtype=np.int32)
    colbase_of = np.full(n, -1, dtype=np.int64)
    tabcol_of = np.full(n, -1, dtype=np.int64)
    buckets = []
    slotline = 0
    tabcol = 0
    i = 0
    while i < len(order):
        w = int(Ws[i])
        j = i
        while j < len(order) and Ws[j] == w:
            j += 1
        nodes = order[i:j]
        nb = len(nodes)
        npl = 128 // w
        nps = (nb + 15) // 16
        L = (nps + npl - 1) // npl
        k = np.arange(nb)
        idx = k // 16
        stripe_of[nodes] = k % 16
        colbase_of[nodes] = (slotline + idx // npl) * 128 + (idx % npl) * w
        tabcol_of[nodes] = tabcol + (idx // npl) * npl + (idx % npl)
        buckets.append(dict(W=w, npl=npl, line0=int(slotline), nlines=int(L),
                            tab0=int(tabcol), nreal=nb))
        slotline += L
        tabcol += L * npl
        i = j
    Lp = (slotline + 3) // 4 * 4
    F = Lp * 128
    N16 = int(tabcol)
    N16pad = (N16 + 511) // 512 * 512
    S = 16 * F

    e1, e2 = firsts, partner
    core_e = np.empty(E, dtype=np.int8)
    core_e[e1] = core_of
    core_e[e2] = core_of
    slot_of_edge = np.empty(E, dtype=np.int64)
    for c in range(N_CORES):
        edges_c = np.flatnonzero(core_e == c)
        d = dst[edges_c].astype(np.int64)
        o = np.argsort(d, kind="stable")
        ds = d[o]
        starts = np.r_[0, np.flatnonzero(np.diff(ds)) + 1]
        runlen = np.diff(np.r_[starts, len(ds)])
        ranks = np.arange(len(ds)) - np.repeat(starts, runlen)
        q = 16 * (colbase_of[ds] + ranks) + stripe_of[ds]
        slot_of_edge[edges_c[o]] = q

    return dict(n=n, F=F, N16=N16, N16pad=N16pad, S=S, buckets=buckets,
                stripe_of=stripe_of, tabcol_of=tabcol_of, W=W,
                core_e=core_e, slot_of_edge=slot_of_edge)


def _build_core_arrays(meta, rev_idx, messages, prior):
    S = meta["S"]; N16pad = meta["N16pad"]; F = meta["F"]
    rev = rev_idx.astype(np.int64)
    logm = np.log(np.maximum(messages, np.float32(EPS))).astype(np.float32)
    logprior = np.log(np.maximum(prior, np.float32(EPS))).astype(np.float32)

    lpT = np.zeros((128, N16pad), dtype=np.float32)
    nodes = np.flatnonzero(meta["tabcol_of"] >= 0)
    s = meta["stripe_of"][nodes]
    tc = meta["tabcol_of"][nodes]
    for k in range(K):
        lpT[s * 8 + k, tc] = logprior[nodes, k]

    cores = []
    for c in range(N_CORES):
        edges_c = np.flatnonzero(meta["core_e"] == c)
        q = meta["slot_of_edge"][edges_c]
        I0 = np.zeros((S, 8), dtype=np.float32)
        I0[q] = logm[edges_c]
        exch = (S + (np.arange(S, dtype=np.int64) & 15)).astype(np.int32)
        exch[q] = meta["slot_of_edge"][rev[edges_c]].astype(np.int32)
        ex = exch.reshape(F // 512, 4, 128, 16)
        ex = np.ascontiguousarray(ex.transpose(0, 2, 1, 3)).reshape(F // 512, 128, 64)
        cores.append(dict(I0=I0, exch=ex))
    return cores, lpT


def _unpermute_output(out_T, meta, prior):
    n = meta["n"]
    out = np.zeros((n, 8), dtype=np.float32)
    nodes = np.flatnonzero(meta["tabcol_of"] >= 0)
    s = meta["stripe_of"][nodes]
    tc = meta["tabcol_of"][nodes]
    for k in range(K):
        out[nodes, k] = out_T[s * 8 + k, tc]
    z = np.flatnonzero(meta["W"] == 0)
    if len(z):
        p = np.maximum(prior[z], np.float32(EPS))
        out[z] = (p / np.maximum(p.sum(-1, keepdims=True), EPS)).astype(np.float32)
    return out


# --------------------------- device program ---------------------------

def _build_program(F, S, N16pad, buckets, iterations):
    import concourse.bacc as bacc
    import concourse.tile as tile
    from concourse import mybir
    from concourse.bass import IndirectOffsetOnAxis

    F32 = mybir.dt.float32
    I32 = mybir.dt.int32
    AF = mybir.ActivationFunctionType
    ALU = mybir.AluOpType
    nmac = F // 512

    nc = bacc.Bacc("TRN2", target_bir_lowering=False, debug=False,
                   num_devices=N_CORES)
    for cval in (LN_EPS, -LN_EPS):
        t = nc.alloc_sbuf_tensor(f"const-float32-{cval}", [128, 1], F32)
        nc.gpsimd.memset(t.ap(), cval)
        nc.const_aps.aps[(F32, cval)] = t.ap()
    nc.all_engine_barrier()
    I0 = nc.dram_tensor("I0", (S, 8), F32, kind="ExternalInput")
    exch = nc.dram_tensor("exch", (nmac, 128, 64), I32, kind="ExternalInput")
    lpT = nc.dram_tensor("lpT", (128, N16pad), F32, kind="ExternalInput")
    psi_blk = nc.dram_tensor("psi_blk", (128, 128), F32, kind="ExternalInput")
    msum_blk = nc.dram_tensor("msum_blk", (128, 128), F32, kind="ExternalInput")
    ones_blk = nc.dram_tensor("ones_blk", (128, 128), F32, kind="ExternalInput")
    ident = nc.dram_tensor("ident", (128, 128), F32, kind="ExternalInput")
    out = nc.dram_tensor("out", (128, N16pad), F32, kind="ExternalOutput")
    groups = [list(range(N_CORES))]

    with tile.TileContext(nc) as tc:
        with tc.tile_pool(name="const", bufs=1) as constp, \
             tc.tile_pool(name="tab", bufs=1) as tabp, \
             tc.tile_pool(name="io", bufs=3) as iop, \
             tc.tile_pool(name="mid", bufs=3) as midp, \
             tc.tile_pool(name="psA", bufs=2, space="PSUM") as psA, \
             tc.tile_pool(name="psB", bufs=1, space="PSUM") as psB, \
             tc.tile_pool(name="dram", bufs=1, space="DRAM") as dramp:

            c_lp = constp.tile([128, N16pad], F32, tag="lp")
            nc.sync.dma_start(c_lp[:], lpT[:])
            c_psi = constp.tile([128, 128], F32, tag="psi")
            nc.sync.dma_start(c_psi[:], psi_blk[:])
            c_msum = constp.tile([128, 128], F32, tag="msum")
            nc.sync.dma_start(c_msum[:], msum_blk[:])
            c_ones = constp.tile([128, 128], F32, tag="ones")
            nc.sync.dma_start(c_ones[:], ones_blk[:])
            c_id = constp.tile([128, 128], F32, tag="id")
            nc.sync.dma_start(c_id[:], ident[:])

            t_tab = tabp.tile([128, N16pad], F32, tag="tab")

            I_a = dramp.tile([S + 16, 8], F32, tag="Ia")
            I_b = dramp.tile([S + 16, 8], F32, tag="Ib")
            cc_in = dramp.tile([128, N16pad], F32, tag="ccin")
            cc_out = dramp.tile([128, N16pad], F32, tag="ccout")
            nc.sync.dma_start(I_a[:S], I0[:])
            nc.sync.dma_start(I_b[:S], I0[:])

            mac_segs = []
            for t4 in range(nmac):
                l0 = t4 * 4
                segs = []
                for b in buckets:
                    lo = max(l0, b["line0"])
                    hi = min(l0 + 4, b["line0"] + b["nlines"])
                    if lo < hi:
                        segs.append((lo - l0, hi - lo, b["npl"], b["W"],
                                     b["tab0"] + (lo - b["line0"]) * b["npl"]))
                mac_segs.append(segs)

            def load_transposed(I_cur, t4):
                ld = iop.tile([128, 4, 128], F32, tag="iload")
                src_ap = I_cur[:S].rearrange(
                    "(t u j c) k -> t j u (c k)", t=nmac, u=4, j=128, c=16
                )[t4]
                nc.sync.dma_start(ld[:], src_ap)
                ps = psA.tile([128, 512], F32, tag="tpsA")
                for u in range(4):
                    nc.tensor.transpose(out=ps[:, u * 128:(u + 1) * 128],
                                        in_=ld[:, u, :], identity=c_id[:])
                return ps

            def seg_ap(ap_, rl, L, npl, W):
                return ap_[:, rl * 128:(rl + L) * 128].rearrange(
                    "p (l c) -> p l c", l=L
                )[:, :, :npl * W].rearrange("p l (m w) -> p l m w", m=npl)

            def reduce_phase(I_cur):
                nc.vector.memset(t_tab[:], 0.0)
                for t4 in range(nmac):
                    if not mac_segs[t4]:
                        continue
                    ps = load_transposed(I_cur, t4)
                    for (rl, L, npl, W, tseg) in mac_segs[t4]:
                        nc.vector.tensor_reduce(
                            out=t_tab[:, tseg:tseg + L * npl],
                            in_=seg_ap(ps, rl, L, npl, W),
                            axis=mybir.AxisListType.X, op=ALU.add)

            def allreduce_and_T():
                nc.gpsimd.dma_start(cc_in[:], t_tab[:])
                nc.gpsimd.collective_compute(
                    "AllReduce", ALU.add, replica_groups=groups,
                    ins=[cc_in[:].opt()], outs=[cc_out[:].opt()])
                nc.sync.dma_start(t_tab[:], cc_out[:])
                nc.vector.tensor_tensor(out=t_tab[:], in0=t_tab[:],
                                        in1=c_lp[:], op=ALU.add)

            def phase_b(I_cur, I_nxt):
                for t4 in range(nmac):
                    ps_in = load_transposed(I_cur, t4)
                    val = midp.tile([128, 512], F32, tag="val")
                    nc.vector.memset(val[:], LN_EPS)
                    for (rl, L, npl, W, tseg) in mac_segs[t4]:
                        nc.vector.scalar_tensor_tensor(
                            out=seg_ap(val, rl, L, npl, W),
                            in0=seg_ap(ps_in, rl, L, npl, W),
                            scalar=-1.0,
                            in1=t_tab[:, tseg:tseg + L * npl].rearrange(
                                "p (l m) -> p l m", l=L
                            ).unsqueeze(3).to_broadcast([128, L, npl, W]),
                            op0=ALU.mult, op1=ALU.add)
                    relu = midp.tile([128, 512], F32, tag="relu")
                    nc.scalar.activation(out=relu[:], in_=val[:], func=AF.Relu,
                                         bias=-LN_EPS)
                    ex = midp.tile([128, 512], F32, tag="exp")
                    nc.scalar.activation(out=ex[:], in_=relu[:], func=AF.Exp,
                                         bias=LN_EPS)
                    ps_w = psB.tile([128, 512], F32, tag="psw")
                    ps_s = psB.tile([128, 512], F32, tag="pss")
                    nc.tensor.matmul(out=ps_w[:], lhsT=c_psi[:], rhs=ex[:],
                                     start=True, stop=True)
                    nc.tensor.matmul(out=ps_s[:], lhsT=c_msum[:], rhs=ex[:],
                                     start=True, stop=True)
                    lw = midp.tile([128, 512], F32, tag="lw")
                    nc.scalar.activation(out=lw[:], in_=ps_w[:], func=AF.Ln)
                    ls = midp.tile([128, 512], F32, tag="ls")
                    nc.scalar.activation(out=ls[:], in_=ps_s[:], func=AF.Ln)
                    o = midp.tile([128, 512], F32, tag="o")
                    nc.vector.tensor_tensor(out=o[:], in0=lw[:], in1=ls[:],
                                            op=ALU.subtract)
                    o2 = midp.tile([128, 512], F32, tag="o2")
                    nc.scalar.activation(out=o2[:], in_=o[:], func=AF.Relu,
                                         bias=-LN_EPS)
                    ps_o = psA.tile([128, 512], F32, tag="tpsO")
                    for u in range(4):
                        nc.tensor.transpose(out=ps_o[:, u * 128:(u + 1) * 128],
                                            in_=o2[:, u * 128:(u + 1) * 128],
                                            identity=c_id[:])
                    ot = midp.tile([128, 512], F32, tag="ot")
                    nc.scalar.activation(out=ot[:], in_=ps_o[:], func=AF.Copy,
                                         bias=LN_EPS)
                    ix = iop.tile([128, 64], I32, tag="ix")
                    nc.sync.dma_start(ix[:], exch[t4])
                    for u in range(4):
                        for s_ in range(16):
                            nc.gpsimd.indirect_dma_start(
                                out=I_nxt[:],
                                out_offset=IndirectOffsetOnAxis(
                                    ap=ix[:, u * 16 + s_:u * 16 + s_ + 1],
                                    axis=0),
                                in_=ot[:, u * 128 + s_ * 8:u * 128 + s_ * 8 + 8],
                                in_offset=None)

            cur, nxt = I_a, I_b
            for it in range(iterations):
                reduce_phase(cur)
                allreduce_and_T()
                phase_b(cur, nxt)
                cur, nxt = nxt, cur

            reduce_phase(cur)
            allreduce_and_T()
            for t in range(N16pad // 512):
                seg = slice(t * 512, (t + 1) * 512)
                r = midp.tile([128, 512], F32, tag="fr")
                nc.scalar.activation(out=r[:], in_=t_tab[:, seg], func=AF.Relu,
                                     bias=-LN_EPS)
                bexp = midp.tile([128, 512], F32, tag="fb")
                nc.scalar.activation(out=bexp[:], in_=r[:], func=AF.Exp,
                                     bias=LN_EPS)
                ps_n = psB.tile([128, 512], F32, tag="psw")
                nc.tensor.matmul(out=ps_n[:], lhsT=c_ones[:], rhs=bexp[:],
                                 start=True, stop=True)
                rec = midp.tile([128, 512], F32, tag="rec")
                nc.vector.reciprocal(out=rec[:], in_=ps_n[:])
                bel = midp.tile([128, 512], F32, tag="bel")
                nc.vector.tensor_tensor(out=bel[:], in0=bexp[:], in1=rec[:],
                                        op=ALU.mult)
                nc.sync.dma_start(out[:, seg], bel[:])

    nc.compile()
    return nc


# --------------------------- entry point ---------------------------

def kernel(prior, messages, potential, src, dst, rev_idx, iterations):
    prior = np.ascontiguousarray(np.asarray(prior, dtype=np.float32))
    messages = np.ascontiguousarray(np.asarray(messages, dtype=np.float32))
    potential = np.asarray(potential, dtype=np.float32)
    src = np.asarray(src, dtype=np.int32)
    dst = np.asarray(dst, dtype=np.int32)
    rev_idx = np.asarray(rev_idx, dtype=np.int32)
    iterations = int(iterations)
    n = prior.shape[0]

    meta = _build_layout(src, dst, rev_idx, n)
    cores, lpT = _build_core_arrays(meta, rev_idx, messages, prior)
    psi = np.exp(potential)
    nc = _build_program(meta["F"], meta["S"], meta["N16pad"], meta["buckets"],
                        iterations)

    eye16 = np.eye(16, dtype=np.float32)
    psi_blk = np.kron(eye16, psi)
    msum_blk = np.kron(eye16, np.outer(psi.sum(axis=1),
                                       np.ones(8, dtype=np.float32)))
    ones_blk = np.kron(eye16, np.ones((8, 8), dtype=np.float32))
    ident = np.eye(128, dtype=np.float32)
    in_maps = [dict(I0=cores[c]["I0"], exch=cores[c]["exch"], lpT=lpT,
                    psi_blk=psi_blk, msum_blk=msum_blk, ones_blk=ones_blk,
                    ident=ident) for c in range(N_CORES)]

    from concourse.bass_utils import run_bass_kernel_spmd
    import time as _time
    _t0 = _time.time()
    res = run_bass_kernel_spmd(nc, in_maps, core_ids=list(range(N_CORES)))
    global LAST_EXEC_WALL_NS
    LAST_EXEC_WALL_NS = int((_time.time() - _t0) * 1e9)
    out_T = res.results[0]["out"]
    return _unpermute_output(out_T, meta, prior)



# revision 2
# speedup vs baseline: 19.0234x; 19.0234x over previous
"""Loopy belief propagation on 8 Trainium2 NeuronCores (Bass/Tile), v2.

Strategy (edge-parallel, per the sharding hint):
  - undirected edge pairs are assigned to the 8 cores with per-(node,core)
    balancing; both directions of a pair live on the same core
  - per core, every directed edge (s->j) gets one "in-slot" in node j's row
    of a padded node-major slot array; node j has W(j) = max_c indeg_c(j)
    slots on every core, so all cores share one global layout
  - per iteration: in_log = strided reduce over each node's slots (regular),
    AllReduce [128, N16] node aggregate across cores, then the per-slot
    update  logm' = lognorm(max(exp(max(T[j]-I, lnEPS)), eps) @ psi)
    computed fully regularly in a transposed-stacked layout
    [128 = 16 stripes x 8 classes, cols]; the new message for the reverse
    edge is delivered to its slot by an indirect-DMA row scatter
  - messages live as uint16 fixed-point (log domain, 2.2e-4 abs step) in
    DRAM; all on-chip math is f32
  - log-prior table is uploaded as a per-core row shard and assembled with
    an on-device AllGather; the final belief table is returned via
    ReduceScatter(max), so each core only downloads 1/8 of the table

All graph-dependent index tables are built on the host (numpy); the device
program is static.
"""
import numpy as np

EPS = 1e-12
LN_EPS = float(np.log(np.float32(EPS)))
N_CORES = 8

# uint16 fixed-point codec for log-domain values in [LN_EPS, 0]:
#   q = ENC_A * x + ENC_B   (x = log value), decode x = q * DEC_S + DEC_B
ENC_A = -2300.0
ENC_B = 8.0
ENC_BIAS_PSO = float(np.float32(ENC_A * LN_EPS + ENC_B))  # bias in (x - LN_EPS) space
DEC_S = float(np.float32(-1.0 / 2300.0))
DEC_B = float(-8.0 * np.float32(-1.0 / 2300.0))

# Problem constants (nn_LoopyBP: n=100000, E=3200000, k=8, iterations=5)
N_NODES = 100_000
E_DIR = 3_200_000
K = 8


# --------------------------- host-side layout ---------------------------

def _assign_cores(u_node, v_node, n, rounds=30, seed=0):
    rng = np.random.default_rng(seed)
    H = len(u_node)
    core_of = rng.integers(0, N_CORES, size=H).astype(np.int8)
    u64 = u_node.astype(np.int64)
    v64 = v_node.astype(np.int64)

    def counts(co):
        c1 = np.bincount(u64 * N_CORES + co, minlength=n * N_CORES)
        c2 = np.bincount(v64 * N_CORES + co, minlength=n * N_CORES)
        return (c1 + c2).reshape(n, N_CORES).astype(np.int32)

    for r in range(rounds):
        cnt = counts(core_of)
        tgt = np.ceil(cnt.sum(1) / N_CORES).astype(np.int32)
        over_u = cnt[u64, core_of] > tgt[u64]
        over_v = cnt[v64, core_of] > tgt[v64]
        bad = np.flatnonzero(over_u | over_v)
        if len(bad) == 0:
            break
        load = cnt[u64[bad]] + cnt[v64[bad]]
        newc = np.argmin(load, axis=1).astype(np.int8)
        take = rng.random(len(bad)) < max(0.15, 0.6 * (0.85 ** r))
        core_of[bad[take]] = newc[take]
    return core_of, counts(core_of)


def _build_layout(src, dst, rev_idx, n):
    E = len(src)
    H = E // 2
    rev = rev_idx.astype(np.int64)
    assert np.all(rev[rev] == np.arange(E)), "rev_idx is not an involution"
    assert not np.any(rev == np.arange(E)), "self-reverse edges unsupported"
    firsts = np.flatnonzero(np.arange(E) < rev)
    assert len(firsts) == H
    partner = rev[firsts]
    pu = dst[partner].astype(np.int64)
    pv = dst[firsts].astype(np.int64)

    core_of, cnt = _assign_cores(pu, pv, n)
    W = cnt.max(axis=1)
    assert W.max() <= 128

    order = np.lexsort((np.arange(n), W))
    order = order[W[order] > 0]
    Ws = W[order]
    stripe_of = np.zeros(n, dtype=np.int32)
    colbase_of = np.full(n, -1, dtype=np.int64)
    tabcol_of = np.full(n, -1, dtype=np.int64)
    buckets = []
    slotline = 0
    tabcol = 0
    i = 0
    while i < len(order):
        w = int(Ws[i])
        j = i
        while j < len(order) and Ws[j] == w:
            j += 1
        nodes = order[i:j]
        nb = len(nodes)
        npl = 128 // w
        nps = (nb + 15) // 16
        L = (nps + npl - 1) // npl
        k = np.arange(nb)
        idx = k // 16
        stripe_of[nodes] = k % 16
        colbase_of[nodes] = (slotline + idx // npl) * 128 + (idx % npl) * w
        tabcol_of[nodes] = tabcol + (idx // npl) * npl + (idx % npl)
        buckets.append(dict(W=w, npl=npl, line0=int(slotline), nlines=int(L),
                            tab0=int(tabcol), nreal=nb))
        slotline += L
        tabcol += L * npl
        i = j
    Lp = (slotline + 3) // 4 * 4
    F = Lp * 128
    N16 = int(tabcol)
    N16pad = (N16 + 511) // 512 * 512
    S = 16 * F

    e1, e2 = firsts, partner
    core_e = np.empty(E, dtype=np.int8)
    core_e[e1] = core_of
    core_e[e2] = core_of
    slot_of_edge = np.empty(E, dtype=np.int64)
    for c in range(N_CORES):
        edges_c = np.flatnonzero(core_e == c)
        d = dst[edges_c].astype(np.int64)
        o = np.argsort(d, kind="stable")
        ds = d[o]
        starts = np.r_[0, np.flatnonzero(np.diff(ds)) + 1]
        runlen = np.diff(np.r_[starts, len(ds)])
        ranks = np.arange(len(ds)) - np.repeat(starts, runlen)
        q = 16 * (colbase_of[ds] + ranks) + stripe_of[ds]
        slot_of_edge[edges_c[o]] = q

    return dict(n=n, F=F, N16=N16, N16pad=N16pad, S=S, buckets=buckets,
                stripe_of=stripe_of, tabcol_of=tabcol_of, W=W,
                core_e=core_e, slot_of_edge=slot_of_edge)


def _build_core_arrays(meta, rev_idx, messages, prior):
    S = meta["S"]; N16pad = meta["N16pad"]; F = meta["F"]
    rev = rev_idx.astype(np.int64)
    logm = np.log(np.maximum(messages, np.float32(EPS))).astype(np.float32)
    logprior = np.log(np.maximum(prior, np.float32(EPS))).astype(np.float32)

    lpT = np.full((128, N16pad), np.uint16(ENC_B), dtype=np.uint16)
    nodes = np.flatnonzero(meta["tabcol_of"] >= 0)
    s = meta["stripe_of"][nodes]
    tc = meta["tabcol_of"][nodes]
    lpq = np.rint(ENC_A * logprior + ENC_B).astype(np.uint16)
    for k in range(K):
        lpT[s * 8 + k, tc] = lpq[nodes, k]

    cores = []
    for c in range(N_CORES):
        edges_c = np.flatnonzero(meta["core_e"] == c)
        q = meta["slot_of_edge"][edges_c]
        I0 = np.full((S, 8), np.uint16(ENC_B), dtype=np.uint16)
        I0[q] = np.rint(ENC_A * logm[edges_c] + ENC_B).astype(np.uint16)
        exch = (S + (np.arange(S, dtype=np.int64) & 15)).astype(np.int32)
        exch[q] = meta["slot_of_edge"][rev[edges_c]].astype(np.int32)
        ex = exch.reshape(F // 512, 4, 128, 16)
        ex = np.ascontiguousarray(ex.transpose(0, 2, 1, 3)).reshape(F // 512, 128, 64)
        cores.append(dict(I0=I0, exch=ex))
    return cores, lpT


def _unpermute_output(out_T, meta, prior):
    n = meta["n"]
    out = np.zeros((n, 8), dtype=np.float32)
    nodes = np.flatnonzero(meta["tabcol_of"] >= 0)
    s = meta["stripe_of"][nodes]
    tc = meta["tabcol_of"][nodes]
    for k in range(K):
        out[nodes, k] = out_T[s * 8 + k, tc]
    z = np.flatnonzero(meta["W"] == 0)
    if len(z):
        p = np.maximum(prior[z], np.float32(EPS))
        out[z] = (p / np.maximum(p.sum(-1, keepdims=True), EPS)).astype(np.float32)
    return out


# --------------------------- device program ---------------------------

def _build_program(F, S, N16pad, buckets, iterations):
    import concourse.bacc as bacc
    import concourse.tile as tile
    from concourse import mybir
    from concourse.bass import IndirectOffsetOnAxis

    F32 = mybir.dt.float32
    F16 = mybir.dt.float16
    U16 = mybir.dt.uint16
    I32 = mybir.dt.int32
    AF = mybir.ActivationFunctionType
    ALU = mybir.AluOpType
    nmac = F // 512
    NSH = N16pad // N_CORES  # 832-style column shard; here: row shard of 16

    nc = bacc.Bacc("TRN2", target_bir_lowering=False, debug=False,
                   num_devices=N_CORES)
    for cval in (LN_EPS, -LN_EPS):
        t = nc.alloc_sbuf_tensor(f"const-float32-{cval}", [128, 1], F32)
        nc.gpsimd.memset(t.ap(), cval)
        nc.const_aps.aps[(F32, cval)] = t.ap()
    nc.all_engine_barrier()
    I0 = nc.dram_tensor("I0", (S, 8), U16, kind="ExternalInput")
    exch = nc.dram_tensor("exch", (nmac, 128, 64), I32, kind="ExternalInput")
    lpT16 = nc.dram_tensor("lpT16", (16, N16pad), U16, kind="ExternalInput")
    psi_blk = nc.dram_tensor("psi_blk", (128, 128), F32, kind="ExternalInput")
    msum_blk = nc.dram_tensor("msum_blk", (128, 128), F32, kind="ExternalInput")
    ones_blk = nc.dram_tensor("ones_blk", (128, 128), F32, kind="ExternalInput")
    ident = nc.dram_tensor("ident", (128, 128), F32, kind="ExternalInput")
    out = nc.dram_tensor("out", (16, N16pad), F32, kind="ExternalOutput")
    groups = [list(range(N_CORES))]

    with tile.TileContext(nc) as tc:
        with tc.tile_pool(name="const", bufs=1) as constp, \
             tc.tile_pool(name="tab", bufs=1) as tabp, \
             tc.tile_pool(name="stage", bufs=1) as stagep, \
             tc.tile_pool(name="io", bufs=3) as iop, \
             tc.tile_pool(name="mid", bufs=3) as midp, \
             tc.tile_pool(name="psA", bufs=2, space="PSUM") as psA, \
             tc.tile_pool(name="psB", bufs=1, space="PSUM") as psB, \
             tc.tile_pool(name="dram", bufs=1, space="DRAM") as dramp:

            c_psi = constp.tile([128, 128], F32, tag="psi")
            nc.sync.dma_start(c_psi[:], psi_blk[:])
            c_msum = constp.tile([128, 128], F32, tag="msum")
            nc.sync.dma_start(c_msum[:], msum_blk[:])
            c_ones = constp.tile([128, 128], F32, tag="ones")
            nc.sync.dma_start(c_ones[:], ones_blk[:])
            c_id = constp.tile([128, 128], F32, tag="id")
            nc.sync.dma_start(c_id[:], ident[:])
        
            # assemble full log-prior table from per-core row shards
            ag_in = dramp.tile([16, N16pad], U16, tag="agin")
            ag_out = dramp.tile([128, N16pad], U16, tag="agout")
            nc.sync.dma_start(ag_in[:], lpT16[:])
            nc.gpsimd.collective_compute(
                "AllGather", ALU.bypass, replica_groups=groups,
                ins=[ag_in[:].opt()], outs=[ag_out[:].opt()])
            c_lp16 = stagep.tile([128, N16pad], U16, tag="lp16")
            nc.sync.dma_start(c_lp16[:], ag_out[:])
            c_lp = constp.tile([128, N16pad], F32, tag="lp")
            nc.scalar.activation(out=c_lp[:], in_=c_lp16[:], func=AF.Copy,
                                 scale=DEC_S, bias=DEC_B)

            # exchange-index table, resident in SBUF for all iterations
            c_ex = constp.tile([128, nmac, 64], I32, tag="ex")
            nc.sync.dma_start(c_ex[:], exch[:].rearrange("t p g -> p t g"))

            t_tab = tabp.tile([128, N16pad], F32, tag="tab")

            I_a = dramp.tile([S + 16, 8], U16, tag="Ia")
            I_b = dramp.tile([S + 16, 8], U16, tag="Ib")
            cc_in = dramp.tile([128, N16pad], F32, tag="ccin")
            cc_out = dramp.tile([128, N16pad], F32, tag="ccout")
            rs_in = dramp.tile([128, N16pad], F32, tag="rsin")
            rs_out = dramp.tile([16, N16pad], F32, tag="rsout")
            nc.sync.dma_start(I_a[:S], I0[:])
            nc.sync.dma_start(I_b[:S], I0[:])

            mac_segs = []
            for t4 in range(nmac):
                l0 = t4 * 4
                segs = []
                for b in buckets:
                    lo = max(l0, b["line0"])
                    hi = min(l0 + 4, b["line0"] + b["nlines"])
                    if lo < hi:
                        segs.append((lo - l0, hi - lo, b["npl"], b["W"],
                                     b["tab0"] + (lo - b["line0"]) * b["npl"]))
                mac_segs.append(segs)

            def load_transposed(I_cur, t4):
                ld = iop.tile([128, 4, 128], U16, tag="iload")
                src_ap = I_cur[:S].rearrange(
                    "(t u j c) k -> t j u (c k)", t=nmac, u=4, j=128, c=16
                )[t4]
                nc.sync.dma_start(ld[:], src_ap)
                ld32 = iop.tile([128, 512], F32, tag="iload32")
                nc.scalar.activation(out=ld32[:],
                                     in_=ld[:].rearrange("p u c -> p (u c)"),
                                     func=AF.Copy, scale=DEC_S, bias=DEC_B)
                ps = psA.tile([128, 512], F32, tag="tpsA")
                for u in range(4):
                    nc.tensor.transpose(out=ps[:, u * 128:(u + 1) * 128],
                                        in_=ld32[:, u * 128:(u + 1) * 128],
                                        identity=c_id[:])
                return ps

            def seg_ap(ap_, rl, L, npl, W):
                return ap_[:, rl * 128:(rl + L) * 128].rearrange(
                    "p (l c) -> p l c", l=L
                )[:, :, :npl * W].rearrange("p l (m w) -> p l m w", m=npl)

            def reduce_phase(I_cur):
                nc.vector.memset(t_tab[:], 0.0)
                for t4 in range(nmac):
                    if not mac_segs[t4]:
                        continue
                    ps = load_transposed(I_cur, t4)
                    for (rl, L, npl, W, tseg) in mac_segs[t4]:
                        nc.vector.tensor_reduce(
                            out=t_tab[:, tseg:tseg + L * npl],
                            in_=seg_ap(ps, rl, L, npl, W),
                            axis=mybir.AxisListType.X, op=ALU.add)

            def allreduce_and_T():
                nc.gpsimd.dma_start(cc_in[:], t_tab[:])
                nc.gpsimd.collective_compute(
                    "AllReduce", ALU.add, replica_groups=groups,
                    ins=[cc_in[:].opt()], outs=[cc_out[:].opt()])
                nc.sync.dma_start(t_tab[:], cc_out[:])
                nc.vector.tensor_tensor(out=t_tab[:], in0=t_tab[:],
                                        in1=c_lp[:], op=ALU.add)

            def phase_b(I_cur, I_nxt):
                for t4 in range(nmac):
                    ps_in = load_transposed(I_cur, t4)
                    val = midp.tile([128, 512], F32, tag="val")
                    nc.vector.memset(val[:], LN_EPS)
                    for (rl, L, npl, W, tseg) in mac_segs[t4]:
                        nc.vector.scalar_tensor_tensor(
                            out=seg_ap(val, rl, L, npl, W),
                            in0=seg_ap(ps_in, rl, L, npl, W),
                            scalar=-1.0,
                            in1=t_tab[:, tseg:tseg + L * npl].rearrange(
                                "p (l m) -> p l m", l=L
                            ).unsqueeze(3).to_broadcast([128, L, npl, W]),
                            op0=ALU.mult, op1=ALU.add)
                    relu = midp.tile([128, 512], F32, tag="relu")
                    nc.scalar.activation(out=relu[:], in_=val[:], func=AF.Relu,
                                         bias=-LN_EPS)
                    ex = midp.tile([128, 512], F32, tag="exp")
                    nc.scalar.activation(out=ex[:], in_=relu[:], func=AF.Exp,
                                         bias=LN_EPS)
                    ps_w = psB.tile([128, 512], F32, tag="psw")
                    ps_s = psB.tile([128, 512], F32, tag="pss")
                    nc.tensor.matmul(out=ps_w[:], lhsT=c_psi[:], rhs=ex[:],
                                     start=True, stop=True)
                    nc.tensor.matmul(out=ps_s[:], lhsT=c_msum[:], rhs=ex[:],
                                     start=True, stop=True)
                    lw = midp.tile([128, 512], F32, tag="lw")
                    nc.scalar.activation(out=lw[:], in_=ps_w[:], func=AF.Ln)
                    ls = midp.tile([128, 512], F32, tag="ls")
                    nc.scalar.activation(out=ls[:], in_=ps_s[:], func=AF.Ln)
                    o = midp.tile([128, 512], F32, tag="o")
                    nc.vector.tensor_tensor(out=o[:], in0=lw[:], in1=ls[:],
                                            op=ALU.subtract)
                    o2 = midp.tile([128, 512], F32, tag="o2")
                    nc.scalar.activation(out=o2[:], in_=o[:], func=AF.Relu,
                                         bias=-LN_EPS)
                    ps_o = psA.tile([128, 512], F32, tag="tpsO")
                    for u in range(4):
                        nc.tensor.transpose(out=ps_o[:, u * 128:(u + 1) * 128],
                                            in_=o2[:, u * 128:(u + 1) * 128],
                                            identity=c_id[:])
                    ot = midp.tile([128, 512], U16, tag="ot")
                    nc.scalar.activation(out=ot[:], in_=ps_o[:], func=AF.Copy,
                                         scale=ENC_A, bias=ENC_BIAS_PSO)
                    for g in range(64):
                        nc.gpsimd.indirect_dma_start(
                            out=I_nxt[:],
                            out_offset=IndirectOffsetOnAxis(
                                ap=c_ex[:, t4, g:g + 1], axis=0),
                            in_=ot[:, g * 8:(g + 1) * 8],
                            in_offset=None)

            cur, nxt = I_a, I_b
            for it in range(iterations):
                reduce_phase(cur)
                allreduce_and_T()
                phase_b(cur, nxt)
                cur, nxt = nxt, cur

            reduce_phase(cur)
            allreduce_and_T()
            for t in range(N16pad // 512):
                seg = slice(t * 512, (t + 1) * 512)
                r = midp.tile([128, 512], F32, tag="fr")
                nc.scalar.activation(out=r[:], in_=t_tab[:, seg], func=AF.Relu,
                                     bias=-LN_EPS)
                bexp = midp.tile([128, 512], F32, tag="fb")
                nc.scalar.activation(out=bexp[:], in_=r[:], func=AF.Exp,
                                     bias=LN_EPS)
                ps_n = psB.tile([128, 512], F32, tag="psw")
                nc.tensor.matmul(out=ps_n[:], lhsT=c_ones[:], rhs=bexp[:],
                                 start=True, stop=True)
                rec = midp.tile([128, 512], F32, tag="rec")
                nc.vector.reciprocal(out=rec[:], in_=ps_n[:])
                bel = midp.tile([128, 512], F32, tag="bel")
                nc.vector.tensor_tensor(out=bel[:], in0=bexp[:], in1=rec[:],
                                        op=ALU.mult)
                nc.sync.dma_start(rs_in[:, seg], bel[:])
            nc.gpsimd.collective_compute(
                "ReduceScatter", ALU.max, replica_groups=groups,
                ins=[rs_in[:].opt()], outs=[rs_out[:].opt()])
            nc.sync.dma_start(out[:], rs_out[:])

    nc.compile()
    return nc


# --------------------------- entry point ---------------------------

def kernel(prior, messages, potential, src, dst, rev_idx, iterations):
    prior = np.ascontiguousarray(np.asarray(prior, dtype=np.float32))
    messages = np.ascontiguousarray(np.asarray(messages, dtype=np.float32))
    potential = np.asarray(potential, dtype=np.float32)
    src = np.asarray(src, dtype=np.int32)
    dst = np.asarray(dst, dtype=np.int32)
    rev_idx = np.asarray(rev_idx, dtype=np.int32)
    iterations = int(iterations)
    n = prior.shape[0]

    meta = _build_layout(src, dst, rev_idx, n)
    cores, lpT = _build_core_arrays(meta, rev_idx, messages, prior)
    psi = np.exp(potential)
    nc = _build_program(meta["F"], meta["S"], meta["N16pad"], meta["buckets"],
                        iterations)

    eye16 = np.eye(16, dtype=np.float32)
    psi_blk = np.kron(eye16, psi)
    msum_blk = np.kron(eye16, np.outer(psi.sum(axis=1),
                                       np.ones(8, dtype=np.float32)))
    ones_blk = np.kron(eye16, np.ones((8, 8), dtype=np.float32))
    ident = np.eye(128, dtype=np.float32)
    in_maps = [dict(I0=cores[c]["I0"], exch=cores[c]["exch"],
                    lpT16=np.ascontiguousarray(lpT[16 * c:16 * (c + 1)]),
                    psi_blk=psi_blk, msum_blk=msum_blk, ones_blk=ones_blk,
                    ident=ident) for c in range(N_CORES)]

    import os as _os
    try:
        import jax as _jax
        _os.makedirs("/tmp/jaxcache", exist_ok=True)
        _jax.config.update("jax_compilation_cache_dir", "/tmp/jaxcache")
        _jax.config.update("jax_persistent_cache_min_compile_time_secs", 0.3)
    except Exception:
        pass
    from concourse.bass_utils import run_bass_kernel_spmd
    import time as _time
    _t0 = _time.time()
    res = run_bass_kernel_spmd(nc, in_maps, core_ids=list(range(N_CORES)))
    global LAST_EXEC_WALL_NS
    LAST_EXEC_WALL_NS = int((_time.time() - _t0) * 1e9)
    out_T = np.concatenate([res.results[c]["out"] for c in range(N_CORES)],
                           axis=0)
    return _unpermute_output(out_T, meta, prior)
